# revision 26
# baseline (speedup 1.0000x reference)
"""Trainium2 Bass kernel for nn_CustomSTFT (STFT -> mag/phase -> iSTFT roundtrip).

Math: the mag/phase roundtrip is the identity, so the module is the LINEAR map
wave = crop(OLA(frames @ A)), A = Wfr.T@Wbr - Wfi.T@Wbi.  A factors EXACTLY:
A[n,m] = w[n] w[m] D(n-m) / 800 with D(d) = sum_{k=0}^{400} cos(pi k d / 400),
and D collapses to D(0)=401, D(even)=1, D(odd)=0.  Therefore

    y[t] = 0.75 x[t] + (1/800) * sum_{frames f containing t}
                       w[t-200f] * S_f^{parity(t)}
    S_f^p = sum_{n: parity p} w[n] x[200f + n]

i.e. a diagonal plus a GLOBAL RANK-8 residual (4 frame offsets x 2 parities).
Per 200-sample block g (of the padded signal, output blocks g=2..2401):

    pass1:  S8[(j,p), f'] = sum_{k par p} w[200j+k] u[k, f']     (matmul M=8)
    S[p, f] = sum_j S8[(j,p), f+j]                               (3 adds)
    pass2:  y[c, g] = 0.75 u[c, g] + sum_{(j',p)} V8[(j',p), c] Sg[(j',p), g]
            Sg[(j',p), g] = S[p, g-j'] (shifted copies, 0 when frame missing)

The two boundary output blocks need a diagonal correction for their missing
frame; that is 2*200 mults per batch row, applied on the host after gather.

Device design notes (per core, 4 batch rows):
 - Both matmul passes run in fp8(e4m3) DoubleRow mode: 0.5 cyc/row and half
   the instructions.  Scales: x is pre-multiplied by 0.75 (so the final mix is
   a plain add), W8 carries 1/16 (keeps S in fp8 normal range), V8 carries
   16/0.75 (so no descale is needed anywhere).  All scale-induced quantization
   errors are ~1e-3 absolute, far under the 2e-2 absmax-relative gate.
 - Only DVE and ACT can read PSUM; Pool (gpsimd) cannot, and Pool has ~1us
   fixed overhead per compute op, so Pool only gets a few full-width ops and
   DMA issues.  Every dma_start costs ~0.6us on its issuing engine, so DMA
   count is minimized and spread over SP/ACT/Pool queues.
"""

import os
import numpy as np
import ml_dtypes

# ---------------- problem constants (hardcoded per contract) ----------------
B, T = 32, 480000
H = 200              # hop / block
NFFT = 800
PAD = 400
N_CORES = 8
BPC = B // N_CORES   # 4 batch rows per core
NB = (T + 2 * PAD) // H      # 2404 input blocks per batch (padded signal)
NF = NB - 3                  # 2401 frames
G = T // H                   # 2400 output blocks per batch
G0 = 2                       # first output block index within padded signal
GRP = 480                    # pass2 output columns per PSUM group
NGRP = G // GRP              # 5
P1B = [0, 482, 962, 1444, 1924, 2404]   # pass1 groups (even starts, <=512)
CC = ((0, 128), (128, 72))   # output-channel (c) chunks over the 200-dim
KD = 100                     # DoubleRow contraction partitions (2*100 = 200)
S_SCALE = 1.0 / 16.0         # folded into W8 (keeps S in fp8 normal range)

_CACHE = {}


# ---------------- host-side weights ----------------
def _host_weights():
    n = np.arange(NFFT)
    w = 0.5 - 0.5 * np.cos(2.0 * np.pi * n / NFFT)  # periodic hann (float64)
    k = np.arange(H)
    W8 = np.zeros((H, 8))
    for j in range(4):
        for p in range(2):
            m = (k % 2) == p
            W8[m, 2 * j + p] = w[200 * j + k[m]]
    V8 = np.zeros((8, H))
    for jp in range(4):
        for p in range(2):
            m = (k % 2) == p
            V8[2 * jp + p, m] = w[200 * jp + k[m]] / NFFT
    dlo = (-0.5 * w[k + 600] ** 2).astype(np.float64)  # g=2: frame f=-1 missing
    dhi = (-0.5 * w[k] ** 2).astype(np.float64)        # g=2401: f=2401 missing
    return W8, V8, dlo, dhi


# ---------------- bass program ----------------
def _build_nc():
    import concourse.bass as bass
    import concourse.mybir as mybir
    from concourse.tile import TileContext
    from concourse.alu_op_type import AluOpType

    DR = mybir.MatmulPerfMode.DoubleRow
    bf16 = mybir.dt.bfloat16
    fp8 = mybir.dt.float8e4
    f32 = mybir.dt.float32

    nc = bass.Bass()
    xt_d = nc.declare_dram_parameter("xt", [H, BPC * NB], bf16, False)
    x2_d = nc.declare_dram_parameter("x2", [KD, 2 * BPC * NB], fp8, False)
    w8_d = nc.declare_dram_parameter("w8", [KD, 2 * 16], fp8, False)
    v8_d = nc.declare_dram_parameter("v8", [4, 2 * 208], fp8, False)
    yt_d = nc.declare_dram_parameter("yt", [H, BPC * G], bf16, True)

    with TileContext(nc) as tc:
        with (
            tc.tile_pool(name="wpool", bufs=1) as wpool,
            tc.tile_pool(name="xpool", bufs=1) as xpool,
            tc.tile_pool(name="spool", bufs=1) as spool,
            tc.tile_pool(name="ypool", bufs=4) as ypool,
            tc.tile_pool(name="rpool", bufs=2) as rpool,
            tc.tile_pool(name="p1", bufs=3, space="PSUM") as p1pool,
            tc.tile_pool(name="p2", bufs=4, space="PSUM") as p2pool,
        ):
            # --- persistent tiles
            w8_t = wpool.tile([KD, 2, 16], fp8, name="w8", tag="w8")
            v8_t = wpool.tile([4, 2, 208], fp8, name="v8", tag="v8")
            xt_t = {}
            for cci, (c0, cn) in enumerate(CC):
                xt_t[cci] = xpool.tile([cn, BPC * NB], bf16,
                                       name=f"xt{cci}", tag=f"xt{cci}")
            x2_t = xpool.tile([KD, 2, BPC * NB], fp8, name="x2", tag="x2")

            nc.sync.dma_start(out=w8_t[:], in_=w8_d[:, :])
            nc.sync.dma_start(out=v8_t[:], in_=v8_d[:, :])
            # x in: fp8 matmul copy first (pass1 is the head of the chain),
            # then the bf16 mix copy, batch-major for early starts
            for b in range(BPC):
                nc.sync.dma_start(
                    out=x2_t[:, 0, b * NB:(b + 1) * NB],
                    in_=x2_d[:, b * NB:(b + 1) * NB])
                nc.sync.dma_start(
                    out=x2_t[:, 1, b * NB:(b + 1) * NB],
                    in_=x2_d[:, BPC * NB + b * NB:(BPC + b + 1) * NB])
            for cci, (c0, cn) in enumerate(CC):
                nc.sync.dma_start(out=xt_t[cci][:], in_=xt_d[c0:c0 + cn, :])

            s8stage = [spool.tile([8, NB], bf16, name=f"s8st{b}", tag=f"s8st{b}")
                       for b in range(BPC)]
            # per-PAIR shift tiles [4 = (b%2,p), NB] so the build ops start at
            # partition 0 and pair 1's pass1 overlaps pair 0's pass2
            s8jh = [[spool.tile([4, NB], bf16, name=f"s8j{h}{j}",
                                tag=f"s8j{h}{j}") for j in range(4)]
                    for h in range(2)]
            t1p = [spool.tile([4, NF], bf16, name=f"t1p{h}", tag=f"t1p{h}")
                   for h in range(2)]
            s_half = [spool.tile([4, NF], fp8, name=f"sh{h}", tag=f"sh{h}")
                      for h in range(2)]
            sg = [spool.tile([4, 2, G], fp8, name=f"sg{b}", tag=f"sg{b}")
                  for b in range(BPC)]

            def emit_pass1(h):
                for gi in range(5):
                    lo, hi = P1B[gi], P1B[gi + 1]
                    for b in (2 * h, 2 * h + 1):
                        ps1 = p1pool.tile([8, hi - lo], f32,
                                          name="ps1", tag="ps1")
                        nc.tensor.matmul(
                            ps1[:], w8_t[:, :, 0:8],
                            x2_t[:, :, b * NB + lo:b * NB + hi],
                            start=True, stop=True, perf_mode=DR,
                        )
                        if gi < 3:
                            nc.scalar.copy(out=s8stage[b][:, lo:hi], in_=ps1[:])
                        else:
                            nc.vector.tensor_copy(out=s8stage[b][:, lo:hi],
                                                  in_=ps1[:])

            def emit_sbuild(h):
                # rearrange (b,(j,p)) -> (j,(b%2,p)) for this pair
                for b in (2 * h, 2 * h + 1):
                    for j in range(4):
                        nc.sync.dma_start(
                            out=s8jh[h][j][2 * (b % 2):2 * (b % 2) + 2, :],
                            in_=s8stage[b][2 * j:2 * j + 2, :])
                # S[p, f] = sum_j S8[(j,p), f+j], bf16 wide adds on DVE
                nc.vector.tensor_tensor(
                    out=t1p[h][:], in0=s8jh[h][0][:, 0:NF],
                    in1=s8jh[h][1][:, 1:1 + NF], op=AluOpType.add)
                nc.vector.tensor_tensor(
                    out=t1p[h][:], in0=t1p[h][:],
                    in1=s8jh[h][2][:, 2:2 + NF], op=AluOpType.add)
                nc.vector.tensor_tensor(
                    out=t1p[h][:], in0=t1p[h][:],
                    in1=s8jh[h][3][:, 3:3 + NF], op=AluOpType.add)
                nc.scalar.copy(out=s_half[h][:], in_=t1p[h][:])  # cast to fp8
                # Sg2[(j'%2,p), j'//2, col] = S[p, col+2-j'] (fp8 bytes)
                for b in (2 * h, 2 * h + 1):
                    sgb = sg[b]
                    nc.vector.memset(sgb[:, :, 0:1], 0.0)
                    nc.vector.memset(sgb[:, :, G - 1:G], 0.0)
                    p0 = 2 * (b % 2)
                    for jp in range(4):
                        dst_lo = max(0, jp - G0)
                        n = min(NF - (G0 - jp + dst_lo), G - dst_lo)
                        nc.gpsimd.dma_start(
                            out=sgb[2 * (jp % 2):2 * (jp % 2) + 2, jp // 2,
                                    dst_lo:dst_lo + n],
                            in_=s_half[h][p0:p0 + 2,
                                          G0 - jp + dst_lo:G0 - jp + dst_lo + n])

            def emit_pass2(h):
                for b in (2 * h, 2 * h + 1):
                    ys0 = ypool.tile([128, G], bf16, name="y0", tag="y0")
                    ys1 = ypool.tile([72, G], bf16, name="y1", tag="y1")
                    rs1 = rpool.tile([72, G], bf16, name="r1", tag="r1")
                    for gi in range(NGRP):
                        o0 = gi * GRP
                        for cci, (c0, cn) in enumerate(CC):
                            ps2 = p2pool.tile([cn, GRP], f32,
                                              name="ps2", tag="ps2")
                            nc.tensor.matmul(
                                ps2[:], v8_t[:, :, c0:c0 + cn],
                                sg[b][:, :, o0:o0 + GRP],
                                start=True, stop=True, perf_mode=DR,
                            )
                            if cci == 0:
                                # DVE mixes straight out of PSUM
                                nc.vector.tensor_tensor(
                                    out=ys0[:, o0:o0 + GRP],
                                    in0=xt_t[0][:, b * NB + G0 + o0:
                                                b * NB + G0 + o0 + GRP],
                                    in1=ps2[:], op=AluOpType.add,
                                )
                            else:
                                # ACT drains (casting to bf16)
                                nc.scalar.copy(out=rs1[:, o0:o0 + GRP],
                                               in_=ps2[:])
                    # all-SBUF bf16 add runs at the DVE fast rate
                    nc.vector.tensor_tensor(
                        out=ys1[:],
                        in0=xt_t[1][:, b * NB + G0:b * NB + G0 + G],
                        in1=rs1[:], op=AluOpType.add,
                    )
                    nc.scalar.dma_start(out=yt_d[0:128, b * G:(b + 1) * G],
                                        in_=ys0[:])
                    nc.gpsimd.dma_start(out=yt_d[128:200, b * G:(b + 1) * G],
                                        in_=ys1[:])

            emit_pass1(0)
            emit_sbuild(0)
            emit_pass1(1)
            emit_pass2(0)
            emit_sbuild(1)
            emit_pass2(1)
    return nc


def _legalize_waits(nc):
    """walrus fuses at most ONE sync-wait into most instructions (and the
    Tile kernel-tail drain gets one per outstanding proc).  Split extras
    into preceding single-wait NoOps on the same engine."""
    import concourse.mybir as mybir

    for f in nc.m.functions:
        for blk in f.blocks:
            new, changed = [], False
            for inst in blk.instructions:
                si = inst.sync_info
                if si is not None and si.on_wait and len(si.on_wait) > 1:
                    waits = list(si.on_wait)
                    for i, w in enumerate(waits[:-1]):
                        nop = mybir.InstNoOp(
                            name=f"{inst.name}-waitsplit{i}", ins=[], outs=[])
                        nop.engine = inst.engine
                        nop.sync_info = mybir.SyncInfo(on_wait=[w], on_update=[])
                        new.append(nop)
                    inst.sync_info = mybir.SyncInfo(
                        on_wait=[waits[-1]], on_update=list(si.on_update or []))
                    changed = True
                new.append(inst)
            if changed:
                blk.instructions = new


def _get_nc():
    if "nc" not in _CACHE:
        nc = _build_nc()
        _legalize_waits(nc)
        _CACHE["nc"] = nc
    return _CACHE["nc"]


# ---------------- host-side data layout ----------------
def _prep_x(x):
    """x [B,T] f32 -> per-core xt [200, BPC*NB] bf16 (pre-scaled by 0.75)
    and x2 [100, 2*BPC*NB] fp8 (DoubleRow k-split: k = i*100 + k2)."""
    xp = np.pad(np.asarray(x, dtype=np.float32) * np.float32(0.75),
                ((0, 0), (PAD, PAD)), mode="edge")
    blocks = xp.reshape(B, NB, H)
    xts, x2s = [], []
    for c in range(N_CORES):
        cb = blocks[c * BPC:(c + 1) * BPC]          # [BPC, NB, H]
        xt = np.ascontiguousarray(cb.transpose(2, 0, 1).reshape(H, BPC * NB))
        xts.append(xt.astype(ml_dtypes.bfloat16))
        x2 = np.ascontiguousarray(
            xt.reshape(2, KD, BPC * NB).transpose(1, 0, 2).reshape(
                KD, 2 * BPC * NB))
        x2s.append(x2.astype(ml_dtypes.float8_e4m3fn))
    return xts, x2s


def _make_in_maps(inputs):
    W8, V8, _, _ = _host_weights()
    xts, x2s = _prep_x(inputs["x"])
    # DoubleRow layouts; scale split: W8 carries 1/16, V8 carries 16/0.75
    w8p = np.zeros((KD, 2, 16))
    w8p[:, :, 0:8] = (W8 * S_SCALE).reshape(2, KD, 8).transpose(1, 0, 2)
    w8 = np.ascontiguousarray(w8p.reshape(KD, 32)).astype(
        ml_dtypes.float8_e4m3fn)
    v8p = np.zeros((4, 2, 208))
    v8p[:, :, 0:H] = (V8 * (16.0 / 0.75)).reshape(2, 4, H).transpose(1, 0, 2)
    v8 = np.ascontiguousarray(v8p.reshape(4, 2 * 208)).astype(
        ml_dtypes.float8_e4m3fn)
    return [{"xt": xts[c], "x2": x2s[c], "w8": w8, "v8": v8}
            for c in range(N_CORES)]


def _finalize(results, x):
    _, _, dlo, dhi = _host_weights()
    out = np.empty((B, T), dtype=np.float32)
    for c in range(N_CORES):
        yt = results[c]["yt"].astype(np.float32).reshape(H, BPC, G)
        out[c * BPC:(c + 1) * BPC] = yt.transpose(1, 2, 0).reshape(BPC, T)
    x = np.asarray(x, dtype=np.float32)
    # boundary blocks: diagonal correction for the one missing frame
    out[:, 0:H] += (dlo[None, :] * x[:, 0:H]).astype(np.float32)
    out[:, T - H:T] += (dhi[None, :] * x[:, T - H:T]).astype(np.float32)
    return out


# ---------------- entry point ----------------
def kernel(x, w_fwd_real, w_fwd_imag, w_bwd_real, w_bwd_imag, **_):
    from concourse.bass_utils import run_bass_kernel_spmd

    in_maps = _make_in_maps({"x": x})
    nc = _get_nc()
    res = run_bass_kernel_spmd(nc, in_maps, list(range(N_CORES)))
    return _finalize(res.results, x)
